# revision 8
# baseline (speedup 1.0000x reference)
"""Morlet CWT (pywt 'morl', widths 1..30) on 8 Trainium2 cores.

Math: for width s, the reference computes
    coef = -sqrt(s) * diff(conv_full(x, filt_s))   cropped center to T
which equals the FIR form
    out[t] = sum_m w_s[m] * x[t + 8s - m],  m in [0, 16s+2)
with w_s = -sqrt(s) * diff1(filt_s)  (filt length 16s+1, diff1 with
zero-padded ends).

Device mapping (per core, 512 of the 4096 (n,c) signals):
  out[t, c] for scale s = sum over signal 128-blocks j of
      W(s, j-m).T @ sigT_block_j          (m = t//128, 8 t-chunks)
  lhsT = W(s, delta) [128k, 128t] precomputed Toeplitz tiles (constants),
  rhs  = sigT block [128k, 512c], every matmul N=512 (full PSUM bank),
  PSUM = 8 banks = 8 t-chunks of one scale, accumulated over delta.
Output DRAM per core: [30, 1024, 512] (s, t, c); host reassembles to
[64, 1024, 64, 30].
"""
import os
import numpy as np

# Defensive: tracing needs an axon NTFF hook this container lacks; a stray
# BASS_TRACE=1 in the environment would crash run_bass_kernel_spmd.
os.environ.setdefault("BASS_NEVER_TRACE", "1")

WIDTHS = list(range(1, 31))
PRECISION = 10
T = 1024
NCORES = 8
CPC = 512           # signals per core
PADROWS = 1536      # 256 zero + 1024 + 256 zero
NBLK = 12           # 1536 / 128

PHASE_CHOICES = (0, 64)   # per-scale signal row phase options (764 vs 834 matmuls/core)
_compiled = None    # (kernel_runner, meta) memo


def _morlet_int_filters(widths):
    xg = np.linspace(-8.0, 8.0, 2 ** PRECISION)
    psi = np.exp(-xg ** 2 / 2.0) * np.cos(5.0 * xg)
    step = xg[1] - xg[0]
    int_psi = np.cumsum(psi) * step
    filts = []
    for s in widths:
        j = np.floor(np.arange(s * (xg[-1] - xg[0]) + 1) / (s * step)).astype(int)
        j = j[j < int_psi.size]
        filts.append(np.ascontiguousarray(int_psi[j][::-1]).astype(np.float32))
    return filts


def _fir_weights():
    """w_s[m] = -sqrt(s) * (f[m] - f[m-1]), m in [0, len(f)+1), f zero-padded."""
    ws = []
    for s, f in zip(WIDTHS, _morlet_int_filters(WIDTHS)):
        f64 = f.astype(np.float64)
        df = np.diff(np.concatenate(([0.0], f64, [0.0])))  # length len(f)+1
        ws.append((-np.sqrt(float(s)) * df).astype(np.float32))
    return ws


def _j_ok(j, phi):
    """Signal block j at row phase phi overlaps the nonzero rows [256,1280)?"""
    lo = phi + 128 * j
    return lo + 128 > 256 and lo < 1280


def _plan():
    """Weight tiles W[(s,delta)], per-scale row phase, matmul schedule.

    Each scale picks the signal-row phase (0 or 64) minimizing its delta
    count: out[t] for t-chunk m accumulates W(s,d).T @ sig_rows[phi+128j :
    phi+128j+128] over d = j - m, where W(s,d)[k,u] = w_s[u-k+O],
    O = 256 + 8s - 128d - phi."""
    ws = _fir_weights()
    wtiles = []          # np [128,128] each
    windex = {}          # (si, delta) -> tile idx
    deltas = {}          # si -> list of deltas
    phases = {}          # si -> 0 | 64
    k = np.arange(128)[:, None]
    u = np.arange(128)[None, :]
    for si, s in enumerate(WIDTHS):
        w = ws[si]
        L = len(w)       # 16s + 2

        def dlist(phi):
            out = []
            for d in range(-3, 7):
                O = 256 + 8 * s - 128 * d - phi
                if O + 127 < 0 or O - 127 > L - 1:
                    continue
                out.append(d)
            return out

        def nmm(phi):
            return sum(1 for m in range(8) for d in dlist(phi)
                       if _j_ok(m + d, phi))

        phi = min(PHASE_CHOICES, key=nmm)
        phases[si] = phi
        dl = []
        for d in dlist(phi):
            O = 256 + 8 * s - 128 * d - phi
            idx = u - k + O
            valid = (idx >= 0) & (idx < L)
            tile = np.where(valid, w[np.clip(idx, 0, L - 1)], 0.0).astype(np.float32)
            windex[(si, d)] = len(wtiles)
            wtiles.append(tile)
            dl.append(d)
        deltas[si] = dl
    return np.stack(wtiles), windex, deltas, phases


def _build_program(reps=None):
    """reps=None: normal kernel. reps=R: timing variant — output goes to an
    Internal DRAM scratch (no download), body wrapped in a For_i(0, R)."""
    import concourse.mybir as mybir
    from concourse import bacc
    from concourse.tile import TileContext

    wts_np, windex, deltas, phases = _plan()
    NW = wts_np.shape[0]
    timing = reps is not None

    nc = bacc.Bacc("TRN2", target_bir_lowering=False, debug=False)
    d_xt = nc.dram_tensor("xt", [PADROWS, CPC], mybir.dt.float32r,
                          kind="ExternalInput")
    d_w = nc.dram_tensor("wts", [NW, 128, 128], mybir.dt.float32r,
                         kind="ExternalInput")
    if timing:
        d_out = nc.dram_tensor("scratch", [len(WIDTHS), T, CPC],
                               mybir.dt.float32, kind="Internal")
        d_tiny = nc.dram_tensor("out", [128, 4], mybir.dt.float32,
                                kind="ExternalOutput")
    else:
        d_out = nc.dram_tensor("out", [len(WIDTHS), T, CPC], mybir.dt.float32,
                               kind="ExternalOutput")

    with TileContext(nc) as tc:
        with tc.tile_pool(name="sigp", bufs=1) as sigp, \
             tc.tile_pool(name="wp", bufs=1) as wp, \
             tc.tile_pool(name="stp", bufs=8) as stp, \
             tc.tile_pool(name="pp", bufs=8, space="PSUM") as pp:
            sig = {}
            for j in range(NBLK):
                t_ = sigp.tile([128, CPC], mybir.dt.float32r, name=f"sig{j}")
                nc.sync.dma_start(t_, d_xt.ap()[128 * j:128 * (j + 1), :])
                sig[(0, j)] = t_
            if any(p == 64 for p in phases.values()):
                for j in range(1, 10):
                    t_ = sigp.tile([128, CPC], mybir.dt.float32r,
                                   name=f"sigb{j}")
                    nc.sync.dma_start(
                        t_, d_xt.ap()[64 + 128 * j:64 + 128 * (j + 1), :])
                    sig[(64, j)] = t_
            wt = [None] * NW
            for si in range(len(WIDTHS)):
                for d in deltas[si]:
                    wi = windex[(si, d)]
                    w_ = wp.tile([128, 128], mybir.dt.float32r, name=f"w{wi}")
                    nc.sync.dma_start(w_, d_w.ap()[wi])
                    wt[wi] = w_

            def body(_iv=None):
              for si in range(len(WIDTHS)):
                for h in range(2):               # t-halves, 4 chunks each
                    ms = [4 * h + q for q in range(4)]
                    phi = phases[si]
                    # usable deltas per chunk (skip all-zero signal blocks)
                    dl = {m: [d for d in deltas[si] if _j_ok(m + d, phi)]
                          for m in ms}
                    ps = {m: pp.tile([128, 512], mybir.dt.float32,
                                     name="ps", tag="ps") for m in ms}
                    for d in deltas[si]:
                        for m in ms:
                            if d not in dl[m]:
                                continue
                            nc.tensor.matmul(
                                ps[m], wt[windex[(si, d)]], sig[(phi, m + d)],
                                start=(d == dl[m][0]), stop=(d == dl[m][-1]))
                    stage = stp.tile([128, 2048], mybir.dt.float32,
                                     name="stage", tag="stage")
                    for q, m in enumerate(ms):
                        dst = stage[:, 512 * q:512 * (q + 1)]
                        if (q + 2 * si + h) % 2 == 0:
                            nc.vector.tensor_copy(dst, ps[m])
                        else:
                            nc.scalar.copy(dst, ps[m])
                    dst = d_out.ap()[si, 512 * h:512 * (h + 1), :]
                    dst = dst.rearrange("(q p) c -> p q c", p=128)
                    src = stage.rearrange("p (q c) -> p q c", q=4)
                    nc.sync.dma_start(dst, src)
                    if timing and si == 0 and h == 0:
                        nc.sync.dma_start(d_tiny.ap(), stage[:, 0:4])

            if timing:
                with tc.For_i(0, reps, 1) as iv:
                    body(iv)
            else:
                body()
    nc.compile()
    return nc, wts_np


def _get_compiled():
    global _compiled
    if _compiled is None:
        nc, wts_np = _build_program()
        _compiled = (nc, wts_np)
    return _compiled


def _core_inmaps(x):
    xt = np.zeros((PADROWS, x.shape[0] * x.shape[2]), np.float32)
    xt[256:256 + T] = x.transpose(1, 0, 2).reshape(T, -1)
    return xt


def _timing_walls(inputs, r_lo=1, r_hi=9, n=10):
    """Wall-clock for timing-variant programs with R=r_lo vs r_hi reps.
    Inputs are staged on device once; diff isolates per-rep HW time."""
    import time
    import jax
    from jax.sharding import Mesh, PartitionSpec, NamedSharding
    from runner import CompiledKernel

    x = np.asarray(inputs["x"], dtype=np.float32)
    xt = _core_inmaps(x)
    walls = {}
    for reps in (r_lo, r_hi):
        nc, wts_np = _build_program(reps=reps)
        k = CompiledKernel(nc, NCORES)
        in_maps = [{"xt": np.ascontiguousarray(xt[:, CPC * c:CPC * (c + 1)]),
                    "wts": wts_np} for c in range(NCORES)]
        per_core = [[np.asarray(m[name]) for name in k.in_names]
                    for m in in_maps]
        concat_in = [np.concatenate([per_core[c][i] for c in range(NCORES)],
                                    axis=0) for i in range(k.n_params)]
        mesh = Mesh(np.asarray(jax.devices()[:NCORES]), ("core",))
        sh = NamedSharding(mesh, PartitionSpec("core"))
        dev_in = [jax.device_put(a, sh) for a in concat_in]
        zeros = [np.zeros((NCORES * z.shape[0], *z.shape[1:]), z.dtype)
                 for z in k.zero_outs]
        ts = []
        for _ in range(n):
            zin = [jax.device_put(z.copy(), sh) for z in zeros]
            t0 = time.perf_counter()
            out = k._fn(*dev_in, *zin)
            jax.block_until_ready(out)
            ts.append(time.perf_counter() - t0)
        walls[reps] = min(ts)
    return r_lo, r_hi, walls[r_lo], walls[r_hi]


def kernel(x, y):
    from concourse import bass_utils

    x = np.asarray(x, dtype=np.float32)          # [64, 1024, 64]
    y_np = np.asarray(y)
    N, T_, C = x.shape
    nc, wts_np = _get_compiled()

    # [T, N*C] padded with 256 zero rows top/bottom
    xt = np.zeros((PADROWS, N * C), np.float32)
    xt[256:256 + T_] = x.transpose(1, 0, 2).reshape(T_, N * C)

    in_maps = []
    for k in range(NCORES):
        in_maps.append({
            "xt": np.ascontiguousarray(xt[:, CPC * k:CPC * (k + 1)]),
            "wts": wts_np,
        })
    res = bass_utils.run_bass_kernel_spmd(nc, in_maps,
                                          core_ids=list(range(NCORES)))
    # per-core [30, 1024, 512] -> out [64, 1024, 64, 30]
    arr = np.stack([r["out"] for r in res.results])      # [8, 30, 1024, 512]
    arr = arr.reshape(NCORES, len(WIDTHS), T_, 8, 64)    # [8, 30, 1024, 8n, 64c]
    out = np.ascontiguousarray(arr.transpose(0, 3, 2, 4, 1)).reshape(
        N, T_, C, len(WIDTHS))
    return (out, y_np)


# revision 9
# speedup vs baseline: 1.0439x; 1.0439x over previous
"""Morlet CWT (pywt 'morl', widths 1..30) on 8 Trainium2 cores.

Math: for width s, the reference computes
    coef = -sqrt(s) * diff(conv_full(x, filt_s))   cropped center to T
which equals the FIR form
    out[t] = sum_m w_s[m] * x[t + 8s - m],  m in [0, 16s+2)
with w_s = -sqrt(s) * diff1(filt_s)  (filt length 16s+1, diff1 with
zero-padded ends).

Device mapping (per core, 512 of the 4096 (n,c) signals):
  out[t, c] for scale s = sum over signal 128-blocks j of
      W(s, j-m).T @ sigT_block_j          (m = t//128, 8 t-chunks)
  lhsT = W(s, delta) [128k, 128t] precomputed Toeplitz tiles (constants),
  rhs  = sigT block [128k, 512c], every matmul N=512 (full PSUM bank),
  PSUM = 8 banks = 8 t-chunks of one scale, accumulated over delta.
Output DRAM per core: [30, 1024, 512] (s, t, c); host reassembles to
[64, 1024, 64, 30].
"""
import os
import numpy as np

# Defensive: tracing needs an axon NTFF hook this container lacks; a stray
# BASS_TRACE=1 in the environment would crash run_bass_kernel_spmd.
os.environ.setdefault("BASS_NEVER_TRACE", "1")

WIDTHS = list(range(1, 31))
PRECISION = 10
T = 1024
NCORES = 8
CPC = 512           # signals per core
PADROWS = 1536      # 256 zero + 1024 + 256 zero
NBLK = 12           # 1536 / 128

PHASE_CHOICES = (0, 64)   # per-scale signal row phase options (764 vs 834 matmuls/core)
_compiled = None    # (kernel_runner, meta) memo


def _morlet_int_filters(widths):
    xg = np.linspace(-8.0, 8.0, 2 ** PRECISION)
    psi = np.exp(-xg ** 2 / 2.0) * np.cos(5.0 * xg)
    step = xg[1] - xg[0]
    int_psi = np.cumsum(psi) * step
    filts = []
    for s in widths:
        j = np.floor(np.arange(s * (xg[-1] - xg[0]) + 1) / (s * step)).astype(int)
        j = j[j < int_psi.size]
        filts.append(np.ascontiguousarray(int_psi[j][::-1]).astype(np.float32))
    return filts


def _fir_weights():
    """w_s[m] = -sqrt(s) * (f[m] - f[m-1]), m in [0, len(f)+1), f zero-padded."""
    ws = []
    for s, f in zip(WIDTHS, _morlet_int_filters(WIDTHS)):
        f64 = f.astype(np.float64)
        df = np.diff(np.concatenate(([0.0], f64, [0.0])))  # length len(f)+1
        ws.append((-np.sqrt(float(s)) * df).astype(np.float32))
    return ws


def _j_ok(j, phi):
    """Signal block j at row phase phi overlaps the nonzero rows [256,1280)?"""
    lo = phi + 128 * j
    return lo + 128 > 256 and lo < 1280


def _plan():
    """Weight tiles W[(s,delta)], per-scale row phase, matmul schedule.

    Each scale picks the signal-row phase (0 or 64) minimizing its delta
    count: out[t] for t-chunk m accumulates W(s,d).T @ sig_rows[phi+128j :
    phi+128j+128] over d = j - m, where W(s,d)[k,u] = w_s[u-k+O],
    O = 256 + 8s - 128d - phi."""
    ws = _fir_weights()
    wtiles = []          # np [128,128] each
    windex = {}          # (si, delta) -> tile idx
    deltas = {}          # si -> list of deltas
    phases = {}          # si -> 0 | 64
    k = np.arange(128)[:, None]
    u = np.arange(128)[None, :]
    for si, s in enumerate(WIDTHS):
        w = ws[si]
        L = len(w)       # 16s + 2

        def dlist(phi):
            out = []
            for d in range(-3, 7):
                O = 256 + 8 * s - 128 * d - phi
                if O + 127 < 0 or O - 127 > L - 1:
                    continue
                out.append(d)
            return out

        def nmm(phi):
            return sum(1 for m in range(8) for d in dlist(phi)
                       if _j_ok(m + d, phi))

        phi = min(PHASE_CHOICES, key=nmm)
        phases[si] = phi
        dl = []
        for d in dlist(phi):
            O = 256 + 8 * s - 128 * d - phi
            idx = u - k + O
            valid = (idx >= 0) & (idx < L)
            tile = np.where(valid, w[np.clip(idx, 0, L - 1)], 0.0).astype(np.float32)
            windex[(si, d)] = len(wtiles)
            wtiles.append(tile)
            dl.append(d)
        deltas[si] = dl
    return np.stack(wtiles), windex, deltas, phases


def _build_program(reps=None):
    """reps=None: normal kernel. reps=R: timing variant — output goes to an
    Internal DRAM scratch (no download), body wrapped in a For_i(0, R)."""
    import concourse.mybir as mybir
    from concourse import bacc
    from concourse.tile import TileContext

    wts_np, windex, deltas, phases = _plan()
    NW = wts_np.shape[0]
    timing = reps is not None

    nc = bacc.Bacc("TRN2", target_bir_lowering=False, debug=False)
    d_xt = nc.dram_tensor("xt", [PADROWS, CPC], mybir.dt.float32r,
                          kind="ExternalInput")
    d_w = nc.dram_tensor("wts", [NW, 128, 128], mybir.dt.float32r,
                         kind="ExternalInput")
    if timing:
        d_out = nc.dram_tensor("scratch", [len(WIDTHS), T, CPC],
                               mybir.dt.float32, kind="Internal")
        d_tiny = nc.dram_tensor("out", [128, 4], mybir.dt.float32,
                                kind="ExternalOutput")
    else:
        d_out = nc.dram_tensor("out", [len(WIDTHS), T, CPC], mybir.dt.float32,
                               kind="ExternalOutput")

    with TileContext(nc) as tc:
        with tc.tile_pool(name="sigp", bufs=1) as sigp, \
             tc.tile_pool(name="wp", bufs=1) as wp, \
             tc.tile_pool(name="stp", bufs=10) as stp, \
             tc.tile_pool(name="pp", bufs=8, space="PSUM") as pp:
            sig = {}
            for j in range(NBLK):
                t_ = sigp.tile([128, CPC], mybir.dt.float32r, name=f"sig{j}")
                nc.sync.dma_start(t_, d_xt.ap()[128 * j:128 * (j + 1), :])
                sig[(0, j)] = t_
            if any(p == 64 for p in phases.values()):
                for j in range(1, 10):
                    t_ = sigp.tile([128, CPC], mybir.dt.float32r,
                                   name=f"sigb{j}")
                    nc.sync.dma_start(
                        t_, d_xt.ap()[64 + 128 * j:64 + 128 * (j + 1), :])
                    sig[(64, j)] = t_
            wt = [None] * NW
            for si in range(len(WIDTHS)):
                for d in deltas[si]:
                    wi = windex[(si, d)]
                    w_ = wp.tile([128, 128], mybir.dt.float32r, name=f"w{wi}")
                    nc.sync.dma_start(w_, d_w.ap()[wi])
                    wt[wi] = w_

            def body(_iv=None):
              for si in range(len(WIDTHS)):
                for h in range(2):               # t-halves, 4 chunks each
                    ms = [4 * h + q for q in range(4)]
                    phi = phases[si]
                    # usable deltas per chunk (skip all-zero signal blocks)
                    dl = {m: [d for d in deltas[si] if _j_ok(m + d, phi)]
                          for m in ms}
                    ps = {m: pp.tile([128, 512], mybir.dt.float32,
                                     name="ps", tag="ps") for m in ms}
                    for d in deltas[si]:
                        for m in ms:
                            if d not in dl[m]:
                                continue
                            nc.tensor.matmul(
                                ps[m], wt[windex[(si, d)]], sig[(phi, m + d)],
                                start=(d == dl[m][0]), stop=(d == dl[m][-1]))
                    stage = stp.tile([128, 2048], mybir.dt.float32,
                                     name="stage", tag="stage")
                    for q, m in enumerate(ms):
                        dst = stage[:, 512 * q:512 * (q + 1)]
                        if (q + 2 * si + h) % 2 == 0:
                            nc.vector.tensor_copy(dst, ps[m])
                        else:
                            nc.scalar.copy(dst, ps[m])
                    dst = d_out.ap()[si, 512 * h:512 * (h + 1), :]
                    dst = dst.rearrange("(q p) c -> p q c", p=128)
                    src = stage.rearrange("p (q c) -> p q c", q=4)
                    nc.sync.dma_start(dst, src)
                    if timing and si == 0 and h == 0:
                        nc.sync.dma_start(d_tiny.ap(), stage[:, 0:4])

            if timing:
                with tc.For_i(0, reps, 1) as iv:
                    body(iv)
            else:
                body()
    nc.compile()
    return nc, wts_np


def _get_compiled():
    global _compiled
    if _compiled is None:
        nc, wts_np = _build_program()
        _compiled = (nc, wts_np)
    return _compiled


def _core_inmaps(x):
    xt = np.zeros((PADROWS, x.shape[0] * x.shape[2]), np.float32)
    xt[256:256 + T] = x.transpose(1, 0, 2).reshape(T, -1)
    return xt


def _timing_walls(inputs, r_lo=1, r_hi=9, n=10):
    """Wall-clock for timing-variant programs with R=r_lo vs r_hi reps.
    Inputs are staged on device once; diff isolates per-rep HW time."""
    import time
    import jax
    from jax.sharding import Mesh, PartitionSpec, NamedSharding
    from runner import CompiledKernel

    x = np.asarray(inputs["x"], dtype=np.float32)
    xt = _core_inmaps(x)
    walls = {}
    for reps in (r_lo, r_hi):
        nc, wts_np = _build_program(reps=reps)
        k = CompiledKernel(nc, NCORES)
        in_maps = [{"xt": np.ascontiguousarray(xt[:, CPC * c:CPC * (c + 1)]),
                    "wts": wts_np} for c in range(NCORES)]
        per_core = [[np.asarray(m[name]) for name in k.in_names]
                    for m in in_maps]
        concat_in = [np.concatenate([per_core[c][i] for c in range(NCORES)],
                                    axis=0) for i in range(k.n_params)]
        mesh = Mesh(np.asarray(jax.devices()[:NCORES]), ("core",))
        sh = NamedSharding(mesh, PartitionSpec("core"))
        dev_in = [jax.device_put(a, sh) for a in concat_in]
        zeros = [np.zeros((NCORES * z.shape[0], *z.shape[1:]), z.dtype)
                 for z in k.zero_outs]
        ts = []
        for _ in range(n):
            zin = [jax.device_put(z.copy(), sh) for z in zeros]
            t0 = time.perf_counter()
            out = k._fn(*dev_in, *zin)
            jax.block_until_ready(out)
            ts.append(time.perf_counter() - t0)
        walls[reps] = min(ts)
    return r_lo, r_hi, walls[r_lo], walls[r_hi]


def kernel(x, y):
    from concourse import bass_utils

    x = np.asarray(x, dtype=np.float32)          # [64, 1024, 64]
    y_np = np.asarray(y)
    N, T_, C = x.shape
    nc, wts_np = _get_compiled()

    # [T, N*C] padded with 256 zero rows top/bottom
    xt = np.zeros((PADROWS, N * C), np.float32)
    xt[256:256 + T_] = x.transpose(1, 0, 2).reshape(T_, N * C)

    in_maps = []
    for k in range(NCORES):
        in_maps.append({
            "xt": np.ascontiguousarray(xt[:, CPC * k:CPC * (k + 1)]),
            "wts": wts_np,
        })
    res = bass_utils.run_bass_kernel_spmd(nc, in_maps,
                                          core_ids=list(range(NCORES)))
    # per-core [30, 1024, 512] -> out [64, 1024, 64, 30]
    arr = np.stack([r["out"] for r in res.results])      # [8, 30, 1024, 512]
    arr = arr.reshape(NCORES, len(WIDTHS), T_, 8, 64)    # [8, 30, 1024, 8n, 64c]
    out = np.ascontiguousarray(arr.transpose(0, 3, 2, 4, 1)).reshape(
        N, T_, C, len(WIDTHS))
    return (out, y_np)
